# revision 20
# baseline (speedup 1.0000x reference)
"""Trainium2 Bass kernel for a 6-layer ViT-style transformer with relative
position bias (B=2, N=1024, D=768, H=12, FF=3072), returning
(final_layernorm_out, feat_after_layer2, feat_after_layer5).

Sharding: data-parallel over batch (2 groups of 4 cores) x tensor-parallel
over heads (3 heads/core) for attention.  The FFN is sequence-sharded: the
attention output projection PSUM also accumulates x/4 + out_b/4, so a single
ReduceScatter (fp32) hands each core `x + attn_delta` for its 256-token
slice; the core then runs the FULL FFN (streamed unsharded weights) on that
slice and a single AllGather (fp32) reassembles the updated residual stream.
Two collectives per layer instead of four all-reduces.

Matmuls are bf16 with fp32 PSUM accumulation; layernorm statistics, softmax
normalization and the residual stream are fp32.  LayerNorm gains/biases are
folded into the following matmul's weights host-side; the relative position
bias is added to the scores via an identity-matmul accumulation into the
scores PSUM; biases enter PSUMs the same way via broadcast bf16 tiles.
"""
import os
import sys
import types

sys.path.insert(0, "/opt/trn_rl_repo")

import numpy as np
import ml_dtypes

import concourse.bass as bass
import concourse.tile as tile
from concourse import bacc, mybir
from concourse.bass_utils import run_bass_kernel_spmd
from concourse.masks import make_identity

BF16 = ml_dtypes.bfloat16

DEPTH = 6
H = 12
D = 768
FF = 4 * D
B = 2
N = 1024
DH = 64
EPS = 1e-5
OUT_INDICES = (2, 5)

N_CORES = 8
TP = 4                      # group size (batch-DP across the two groups)
NH = H // TP                # local heads = 3
NT = N // 128               # token tiles = 8
ND = D // 128               # D tiles = 6
NFF = FF // 128             # full ff tiles = 24
NLT = NT // TP              # local token tiles per core = 2
NCH = 2                     # query chunks (PSUM-bank sizing)
TCH = N // NCH              # 512

F32 = mybir.dt.float32
BF = mybir.dt.bfloat16
FP16 = mybir.dt.float16

REPLICA_GROUPS = [[0, 1, 2, 3], [4, 5, 6, 7]]


def _bn_ln_stats(nc, pool, x_ap, eps_sb):
    """Return (mean, inv_std) [128,1] f32 APs for layernorm over free dim."""
    sub = 256  # gcd(BN_STATS_FMAX=512, 768)
    nsub = x_ap.shape[-1] // sub
    xg = x_ap.rearrange("p (s f) -> p s f", f=sub)
    stats = pool.tile([128, nsub, nc.vector.BN_STATS_DIM], F32, tag="ln_stats")
    for s in range(nsub):
        nc.vector.bn_stats(out=stats[:, s, :], in_=xg[:, s, :])
    mv = pool.tile([128, nc.vector.BN_AGGR_DIM], F32, tag="ln_mv")
    nc.vector.bn_aggr(out=mv, in_=stats)
    mean = mv[:, 0:1]
    var = mv[:, 1:2]
    std = pool.tile([128, 1], F32, tag="ln_std")
    nc.scalar.activation(out=std, in_=var,
                         func=mybir.ActivationFunctionType.Sqrt,
                         bias=eps_sb, scale=1.0)
    inv = pool.tile([128, 1], F32, tag="ln_inv")
    nc.vector.reciprocal(out=inv, in_=std)
    return mean, inv


def _bcast_dma(nc, dst, src_row, engine=None):
    """DMA a [X] DRAM row broadcast across dst's partitions."""
    eng = engine or nc.sync
    eng.dma_start(out=dst, in_=bass.AP(
        tensor=src_row.tensor, offset=src_row.offset,
        ap=[[0, dst.shape[0]]] + src_row.ap))


def build_nc():
    nc = bacc.Bacc("TRN2", target_bir_lowering=False, debug=False,
                   num_devices=N_CORES)

    dt_in = lambda n, s, d: nc.dram_tensor(n, s, d, kind="ExternalInput").ap()
    dt_out = lambda n, s, d: nc.dram_tensor(n, s, d, kind="ExternalOutput").ap()

    x_d = dt_in("x", [N, D], FP16)
    wqk_d = dt_in("wqk", [DEPTH, 128, ND * NH * 128], BF)
    bqk_d = dt_in("bqk", [DEPTH, 128, NH], F32)
    wv_d = dt_in("wv", [DEPTH, 128, ND * NH * DH], BF)
    bv_d = dt_in("bv", [DEPTH, DH, NH], F32)
    wo_d = dt_in("wo", [DEPTH, DH, NH * D], BF)
    ob4_d = dt_in("ob4", [DEPTH, D], BF)     # out_b / TP, bf16
    w1_d = dt_in("w1", [DEPTH, NFF, 128, ND * 128], BF)
    b1_d = dt_in("b1", [DEPTH, 128, NFF], F32)
    w2_d = dt_in("w2", [DEPTH, NFF, 128, D], BF)
    b2_d = dt_in("b2", [DEPTH, D], BF)       # full ff2_b, bf16
    eb_d = dt_in("eb", [NH, N, N], BF)
    gfbf_d = dt_in("gfbf", [2, D], F32)

    y0_d = dt_out("y0", [N, D], F32)   # final LN out
    y1_d = dt_out("y1", [N, D], FP16)   # x after layer 2
    y2_d = dt_out("y2", [N, D], FP16)   # x after layer 5

    with tile.TileContext(nc) as tc:
        _body(nc, tc, x_d, wqk_d, bqk_d, wv_d, bv_d, wo_d, ob4_d, w1_d, b1_d,
              w2_d, b2_d, eb_d, gfbf_d, y0_d, y1_d, y2_d)
    nc.compile()
    return nc


def _body(nc, tc, x_d, wqk_d, bqk_d, wv_d, bv_d, wo_d, ob4_d, w1_d, b1_d,
          w2_d, b2_d, eb_d, gfbf_d, y0_d, y1_d, y2_d):
    from contextlib import ExitStack
    ctx = ExitStack()
    with ctx:
        const = ctx.enter_context(tc.tile_pool(name="const", bufs=1))
        xres = ctx.enter_context(tc.tile_pool(name="xres", bufs=1))
        wpool = ctx.enter_context(tc.tile_pool(name="w", bufs=2))
        wsmall = ctx.enter_context(tc.tile_pool(name="wsmall", bufs=1))
        wfp = ctx.enter_context(tc.tile_pool(name="wfp", bufs=4))
        act = ctx.enter_context(tc.tile_pool(name="act", bufs=1))
        small = ctx.enter_context(tc.tile_pool(name="small", bufs=3))
        attnp = ctx.enter_context(tc.tile_pool(name="attn", bufs=6))
        ebp = ctx.enter_context(tc.tile_pool(name="ebp", bufs=12))
        xqp = ctx.enter_context(tc.tile_pool(name="xqp", bufs=4))
        parp = ctx.enter_context(tc.tile_pool(name="parp", bufs=4))
        xnwp = ctx.enter_context(tc.tile_pool(name="xnwp", bufs=2))
        foutp = ctx.enter_context(tc.tile_pool(name="foutp", bufs=2))
        psum = ctx.enter_context(tc.tile_pool(name="ps", bufs=3, space="PSUM"))
        psum_o = ctx.enter_context(tc.tile_pool(name="pso", bufs=1, space="PSUM"))
        psum_p = ctx.enter_context(tc.tile_pool(name="psp", bufs=4, space="PSUM"))
        dram = ctx.enter_context(tc.tile_pool(name="dram", bufs=2, space="DRAM"))

        ident = const.tile([128, 128], BF)
        make_identity(nc, ident)
        eps_sb = const.tile([128, 1], F32)
        nc.vector.memset(eps_sb, EPS)
        ones_r = const.tile([1, DH], BF)
        nc.vector.memset(ones_r, 1.0)

        # resident activations: x[p, t, d] with token = t*128 + p
        x_sb = xres.tile([128, NT, D], FP16)
        nc.sync.dma_start(out=x_sb, in_=x_d.rearrange("(t p) d -> p t d", p=128))

        # final-LN gain/bias broadcast tiles
        gf_bc = const.tile([128, D], F32)
        bf_bc = const.tile([128, D], F32)
        _bcast_dma(nc, gf_bc, gfbf_d[0])
        _bcast_dma(nc, bf_bc, gfbf_d[1])

        hT_sb = act.tile([128, ND, N], BF)           # transposed LN1 output
        h2T_sb = act.tile([128, ND, NLT * 128], BF)  # transposed LN2 (local)
        qk_sb = act.tile([128, NH, N], BF)           # rows 0-63 qT, 64-127 kT
        k_sb = act.tile([DH, NH, N], BF)             # kT shifted to partition 0
        v_sb = act.tile([128, NT, NH, DH + 8], BF)   # token-major v + ones col
        ot_sb = act.tile([DH, NH, N], BF)            # normalized o^T per head
        ffT_sb = act.tile([128, NFF, NLT * 128], BF)  # gelu(ff1), local slice
        rsl_sb = act.tile([128, NLT, D], FP16)       # reduce-scatter output
        r_sb = act.tile([1, N], F32)

        nc.vector.memset(v_sb[:, :, :, DH:DH + 1], 1.0)

        # warm-up collective: absorbs the ~40us cold-start cost of the CC
        # path before layer 0's ReduceScatter hits the critical path
        wu_in = dram.tile([TP, 128, 16], F32, tag="wu_in")
        wu_out = dram.tile([1, 128, 16], F32, tag="wu_out")
        wu_sb = small.tile([128, 16], F32, tag="wu")
        nc.vector.memset(wu_sb, 0.0)
        for _wi in range(TP):
            nc.sync.dma_start(out=wu_in[_wi], in_=wu_sb)
        nc.gpsimd.collective_compute(
            "ReduceScatter", mybir.AluOpType.add,
            replica_groups=REPLICA_GROUPS,
            ins=[wu_in[:].opt()], outs=[wu_out[:].opt()])
        nc.sync.dma_start(out=wu_sb, in_=wu_out[0])

        def ln_transpose(src_ap, dstT, dst_col):
            mean, inv = _bn_ln_stats(nc, small, src_ap, eps_sb)
            ht = small.tile([128, D], BF, tag="h")
            nc.vector.tensor_scalar(
                out=ht, in0=src_ap, scalar1=mean, scalar2=inv,
                op0=mybir.AluOpType.subtract, op1=mybir.AluOpType.mult)
            for dt in range(ND):
                trp = psum.tile([128, 128], BF, tag="mm")
                nc.tensor.transpose(
                    trp, ht[:, dt * 128:(dt + 1) * 128], ident)
                nc.vector.tensor_copy(
                    out=dstT[:, dt, dst_col:dst_col + 128], in_=trp)

        for layer in range(DEPTH):
            # ---- per-layer attention weights (gpsimd DMA queue) ----
            wqk = wpool.tile([128, ND * NH, 128], BF, tag="wqk")
            nc.gpsimd.dma_start(out=wqk, in_=wqk_d[layer].rearrange(
                "p (dh c) -> p dh c", c=128))
            wv = wpool.tile([128, ND * NH, DH], BF, tag="wv")
            nc.gpsimd.dma_start(out=wv, in_=wv_d[layer].rearrange(
                "p (dh c) -> p dh c", c=DH))
            wo = wpool.tile([DH, NH, D], BF, tag="wo")
            nc.gpsimd.dma_start(out=wo, in_=wo_d[layer].rearrange(
                "p (h c) -> p h c", c=D))
            bqk = wsmall.tile([128, NH], F32, tag="bqk")
            nc.gpsimd.dma_start(out=bqk, in_=bqk_d[layer])
            bv = wsmall.tile([DH, NH], F32, tag="bv")
            nc.gpsimd.dma_start(out=bv, in_=bv_d[layer])
            b1t = wsmall.tile([128, NFF], F32, tag="b1")
            nc.gpsimd.dma_start(out=b1t, in_=b1_d[layer])
            ob4_bc = wsmall.tile([128, D], BF, tag="ob4")
            _bcast_dma(nc, ob4_bc, ob4_d[layer], nc.gpsimd)
            b2_bc = wsmall.tile([128, D], BF, tag="b2")
            _bcast_dma(nc, b2_bc, b2_d[layer], nc.gpsimd)

            # ---- LN1 + transpose to hT (all 8 token tiles) ----
            for t in range(NT):
                ln_transpose(x_sb[:, t, :], hT_sb, t * 128)

            # ---- qkT per head: [128, N] (rows 0-63 q^T, 64-127 k^T) ----
            for h in range(NH):
                for c in range(NCH):
                    mm = psum.tile([128, TCH], F32, tag="mm")
                    for dt in range(ND):
                        nc.tensor.matmul(
                            mm, wqk[:, dt * NH + h, :],
                            hT_sb[:, dt, c * TCH:(c + 1) * TCH],
                            start=(dt == 0), stop=(dt == ND - 1))
                    nc.vector.tensor_scalar_add(
                        out=qk_sb[:, h, c * TCH:(c + 1) * TCH], in0=mm,
                        scalar1=bqk[:, h:h + 1])
                    nc.sync.dma_start(
                        out=k_sb[:, h, c * TCH:(c + 1) * TCH],
                        in_=qk_sb[DH:128, h, c * TCH:(c + 1) * TCH])

            # ---- v token-major ----
            for t in range(NT):
                for h in range(NH):
                    mmv = psum.tile([128, TCH], F32, tag="mm")
                    for dt in range(ND):
                        nc.tensor.matmul(
                            mmv[:, 0:DH], hT_sb[:, dt, t * 128:(t + 1) * 128],
                            wv[:, dt * NH + h, :],
                            start=(dt == 0), stop=(dt == ND - 1))
                    nc.vector.tensor_copy(out=v_sb[:, t, h, 0:DH],
                                          in_=mmv[:, 0:DH])

            # ---- attention per head (chunks sequential; eb preloaded) ----
            for h in range(NH):
                ebts = []
                for kt in range(NT):
                    ebt = ebp.tile([128, N], BF, tag="eb")
                    nc.scalar.dma_start(
                        out=ebt, in_=eb_d[h, kt * 128:(kt + 1) * 128, :])
                    ebts.append(ebt)
                for c in range(NCH):
                    sl = slice(c * TCH, (c + 1) * TCH)
                    oc = psum_o.tile([DH + 1, TCH], F32, tag="oacc")
                    for kt in range(NT):
                        sc = psum.tile([128, TCH], F32, tag="mm")
                        nc.tensor.matmul(
                            sc, k_sb[:, h, kt * 128:(kt + 1) * 128],
                            qk_sb[0:DH, h, sl],
                            start=True, stop=False)
                        nc.tensor.matmul(
                            sc, ident, ebts[kt][:, sl],
                            start=False, stop=True)
                        at = attnp.tile([128, TCH], BF, tag="attn")
                        nc.scalar.activation(
                            out=at, in_=sc,
                            func=mybir.ActivationFunctionType.Exp)
                        nc.tensor.matmul(
                            oc, v_sb[:, kt, h, 0:DH + 1], at,
                            start=(kt == 0), stop=(kt == NT - 1))
                    # normalize: recip rowsum, PE broadcast, scale + v-bias
                    rb16 = small.tile([1, TCH], BF, tag="rb16")
                    nc.vector.reciprocal(out=r_sb[:, sl], in_=oc[DH:DH + 1, :])
                    nc.vector.tensor_copy(out=rb16, in_=r_sb[:, sl])
                    ou = small.tile([DH, TCH], F32, tag="ou")
                    nc.vector.tensor_copy(out=ou, in_=oc[0:DH, :])
                    bcp = psum.tile([DH, TCH], F32, tag="mm")
                    nc.tensor.matmul(bcp, ones_r, rb16, start=True, stop=True)
                    nc.vector.tensor_mul(out=ot_sb[:, h, sl], in0=ou, in1=bcp)
                    nc.vector.tensor_scalar_add(
                        out=ot_sb[:, h, sl], in0=ot_sb[:, h, sl],
                        scalar1=bv[:, h:h + 1])

            # ---- out-proj (+ x/4 + out_b/4) -> ReduceScatter ----
            cc_in = dram.tile([NT, 128, D], FP16, tag="cc_in")
            rs_out = dram.tile([NLT, 128, D], FP16, tag="rs_out")
            for t in range(NT):
                xq = xqp.tile([128, D], F32, tag="xq")
                nc.scalar.mul(out=xq, in_=x_sb[:, t, :], mul=1.0 / TP)
                par = parp.tile([128, D], FP16, tag="par")
                for n0, n1 in ((0, 384), (384, 768)):
                    po = psum_p.tile([128, 384], F32, tag="out")
                    for h in range(NH):
                        nc.tensor.matmul(
                            po, ot_sb[:, h, t * 128:(t + 1) * 128],
                            wo[:, h, n0:n1],
                            start=(h == 0), stop=False)
                    nc.tensor.matmul(po, ident, ob4_bc[:, n0:n1],
                                     start=False, stop=True)
                    nc.vector.tensor_add(
                        out=par[:, n0:n1], in0=po, in1=xq[:, n0:n1])
                nc.sync.dma_start(out=cc_in[t], in_=par)
            nc.gpsimd.collective_compute(
                "ReduceScatter", mybir.AluOpType.add,
                replica_groups=REPLICA_GROUPS,
                ins=[cc_in[:].opt()], outs=[rs_out[:].opt()])
            for tl in range(NLT):
                nc.sync.dma_start(out=rsl_sb[:, tl, :], in_=rs_out[tl])

            # ---- local FFN on this core's token slice ----
            for tl in range(NLT):
                ln_transpose(rsl_sb[:, tl, :], h2T_sb, tl * 128)
            po2 = []
            for _tl in range(NLT):
                row = []
                for _ci in range(2):
                    po2t = psum_p.tile([128, 384], F32, tag="out")
                    row.append(po2t)
                po2.append(row)
            for f in range(NFF):
                wf1 = wfp.tile([128, ND, 128], BF, tag="wf1")
                nc.gpsimd.dma_start(out=wf1, in_=w1_d[layer, f].rearrange(
                    "p (dt c) -> p dt c", c=128))
                mmf = psum.tile([128, NLT * 128], F32, tag="mm")
                for dt in range(ND):
                    nc.tensor.matmul(
                        mmf, wf1[:, dt, :], h2T_sb[:, dt, :],
                        start=(dt == 0), stop=(dt == ND - 1))
                nc.scalar.activation(
                    out=ffT_sb[:, f, :], in_=mmf,
                    func=mybir.ActivationFunctionType.Gelu,
                    bias=b1t[:, f:f + 1])
                wf2 = wfp.tile([128, D], BF, tag="wf2")
                nc.gpsimd.dma_start(out=wf2, in_=w2_d[layer, f])
                for tl in range(NLT):
                    for ci, (n0, n1) in enumerate(((0, 384), (384, 768))):
                        nc.tensor.matmul(
                            po2[tl][ci], ffT_sb[:, f, tl * 128:(tl + 1) * 128],
                            wf2[:, n0:n1],
                            start=(f == 0), stop=False)
            ag_in = dram.tile([NLT, 128, D], FP16, tag="ag_in")
            ag_out = dram.tile([NT, 128, D], FP16, tag="ag_out")
            for tl in range(NLT):
                xnw = xnwp.tile([128, D], FP16, tag="xnw")
                for ci, (n0, n1) in enumerate(((0, 384), (384, 768))):
                    nc.tensor.matmul(po2[tl][ci], ident, b2_bc[:, n0:n1],
                                     start=False, stop=True)
                    nc.vector.tensor_add(
                        out=xnw[:, n0:n1], in0=po2[tl][ci],
                        in1=rsl_sb[:, tl, n0:n1])
                nc.sync.dma_start(out=ag_in[tl], in_=xnw)
            nc.gpsimd.collective_compute(
                "AllGather", mybir.AluOpType.bypass,
                replica_groups=REPLICA_GROUPS,
                ins=[ag_in[:].opt()], outs=[ag_out[:].opt()])
            for t in range(NT):
                nc.sync.dma_start(out=x_sb[:, t, :], in_=ag_out[t])

            if layer in OUT_INDICES:
                y_d = y1_d if layer == OUT_INDICES[0] else y2_d
                nc.sync.dma_start(
                    out=y_d.rearrange("(t p) d -> p t d", p=128), in_=x_sb)

        # ---- final layernorm ----
        for t in range(NT):
            mean, inv = _bn_ln_stats(nc, small, x_sb[:, t, :], eps_sb)
            o = foutp.tile([128, D], F32, tag="fout")
            nc.vector.tensor_scalar(
                out=o, in0=x_sb[:, t, :], scalar1=mean, scalar2=inv,
                op0=mybir.AluOpType.subtract, op1=mybir.AluOpType.mult)
            nc.vector.tensor_mul(out=o, in0=o, in1=gf_bc)
            nc.vector.tensor_add(out=o, in0=o, in1=bf_bc)
            nc.sync.dma_start(
                out=y0_d.rearrange("(t p) d -> p t d", p=128)[:, t, :], in_=o)


# ---------------------------------------------------------------------------
# host side
# ---------------------------------------------------------------------------

def _prep_core_inputs(x, rel_bias, ln1_g, ln1_b, qkv_w, out_w, out_b, ln2_g,
                      ln2_b, ff1_w, ff1_b, ff2_w, ff2_b, lnf_g, lnf_b, core):
    """Build the per-core input map (host-side folds + layouts)."""
    x = np.asarray(x); rel_bias = np.asarray(rel_bias)
    ln1_g = np.asarray(ln1_g); ln1_b = np.asarray(ln1_b)
    qkv_w = np.asarray(qkv_w); out_w = np.asarray(out_w)
    out_b = np.asarray(out_b); ln2_g = np.asarray(ln2_g)
    ln2_b = np.asarray(ln2_b); ff1_w = np.asarray(ff1_w)
    ff1_b = np.asarray(ff1_b); ff2_w = np.asarray(ff2_w)
    ff2_b = np.asarray(ff2_b); lnf_g = np.asarray(lnf_g)
    lnf_b = np.asarray(lnf_b)

    b = core // TP          # batch element
    r = core % TP           # tp rank
    heads = list(range(r * NH, (r + 1) * NH))
    scale = DH ** -0.5

    wqk = np.empty((DEPTH, ND, NH, 128, 128), np.float32)
    bqk = np.empty((DEPTH, 128, NH), np.float32)
    wv = np.empty((DEPTH, ND, NH, 128, DH), np.float32)
    bv = np.empty((DEPTH, DH, NH), np.float32)
    wo = np.empty((DEPTH, DH, NH, D), np.float32)
    w1 = np.empty((DEPTH, NFF, 128, ND, 128), np.float32)
    b1 = np.empty((DEPTH, 128, NFF), np.float32)
    w2 = np.empty((DEPTH, NFF, 128, D), np.float32)

    for l in range(DEPTH):
        g1 = ln1_g[l][:, None]
        for hi, h in enumerate(heads):
            wq = qkv_w[l][:, h * DH:(h + 1) * DH] * g1 * scale
            wk = qkv_w[l][:, D + h * DH:D + (h + 1) * DH] * g1
            wvv = qkv_w[l][:, 2 * D + h * DH:2 * D + (h + 1) * DH] * g1
            wqk_h = np.concatenate([wq, wk], axis=1)        # [768, 128]
            wqk[l, :, hi] = wqk_h.reshape(ND, 128, 128)
            bqk[l, :, hi] = np.concatenate([ln1_b[l] @ wq, ln1_b[l] @ wk])
            wv[l, :, hi] = wvv.reshape(ND, 128, DH)
            bv[l, :, hi] = ln1_b[l] @ wvv
            wo[l, :, hi, :] = out_w[l][h * DH:(h + 1) * DH, :]
        w1l = ff1_w[l] * ln2_g[l][:, None]                  # [768, 3072]
        # w1[l, f, p, dt, c] = w1l[dt*128+p, f*128+c]
        w1[l] = w1l.reshape(ND, 128, NFF, 128).transpose(2, 1, 0, 3)
        b1[l] = (ff1_b[l] + ln2_b[l] @ w1l).reshape(NFF, 128).T
        w2[l] = ff2_w[l].reshape(NFF, 128, D)

    eb = rel_bias[heads].transpose(0, 2, 1)   # [NH, keys, queries]

    return {
        "x": np.ascontiguousarray(x[b].astype(np.float16)),
        "wqk": np.ascontiguousarray(wqk.transpose(0, 3, 1, 2, 4).reshape(
            DEPTH, 128, ND * NH * 128).astype(BF16)),
        "bqk": np.ascontiguousarray(bqk),
        "wv": np.ascontiguousarray(wv.transpose(0, 3, 1, 2, 4).reshape(
            DEPTH, 128, ND * NH * DH).astype(BF16)),
        "bv": np.ascontiguousarray(bv),
        "wo": np.ascontiguousarray(wo.reshape(DEPTH, DH, NH * D).astype(BF16)),
        "ob4": np.ascontiguousarray((out_b / TP).astype(BF16)),
        "w1": np.ascontiguousarray(w1.reshape(
            DEPTH, NFF, 128, ND * 128).astype(BF16)),
        "b1": np.ascontiguousarray(b1),
        "w2": np.ascontiguousarray(w2.astype(BF16)),
        "b2": np.ascontiguousarray(ff2_b.astype(BF16)),
        "eb": np.ascontiguousarray(eb.astype(BF16)),
        "gfbf": np.ascontiguousarray(np.stack([lnf_g, lnf_b]), np.float32),
    }


def _install_ntff_hook():
    """Make run_bass_kernel_spmd(trace=True) work: register the axon NTFF
    profile hook that the image's antenv package is missing."""
    try:
        import antenv.axon_hooks  # noqa: F401
        return
    except ImportError:
        pass
    try:
        from trn_agent_boot.trn_boot import _ntff_profile_via_ctypes
        import antenv
        hook = _ntff_profile_via_ctypes("/opt/axon/libaxon_pjrt.so")
        mod = types.ModuleType("antenv.axon_hooks")
        mod.get_axon_ntff_profile_hook = lambda: hook
        mod.set_axon_ntff_profile_hook = lambda h: None
        sys.modules["antenv.axon_hooks"] = mod
        antenv.axon_hooks = mod
    except Exception:
        pass


_NC_CACHE = {}
LAST_RESULT = None


def kernel(**inputs):
    global LAST_RESULT
    if "nc" not in _NC_CACHE:
        _NC_CACHE["nc"] = build_nc()
    nc = _NC_CACHE["nc"]

    in_maps = [_prep_core_inputs(**inputs, core=i) for i in range(N_CORES)]
    trace = bool(int(os.environ.get("KERNEL_TRACE", "0")))
    if trace:
        _install_ntff_hook()
    res = run_bass_kernel_spmd(nc, in_maps, core_ids=list(range(N_CORES)),
                               trace=trace)
    LAST_RESULT = res

    kside = int(round(N ** 0.5))
    xf = np.stack([res.results[0]["y0"], res.results[TP]["y0"]])
    feats = []
    for key in ("y1", "y2"):
        f = np.stack([res.results[0][key], res.results[TP][key]]).astype(np.float32)
        feats.append(np.ascontiguousarray(
            f.reshape(B, kside, kside, D).transpose(0, 3, 1, 2)))
    return (xf, *feats)
